# revision 19
# baseline (speedup 1.0000x reference)
"""RGCN (mean-aggregation) message-passing kernel for 8 Trainium2 NeuronCores.

Problem shapes (hardcoded):
  B=16, L=512, H=256, R=8, E=524288, N = B*2*L = 16384 nodes.

Strategy v3 (dst-sharded, no collectives):
  - Node features x = concat(input_s, input_a) -> [N, H] fp16 table in HBM,
    replicated per core. Core c owns dst nodes [2048c, 2048(c+1)); segments
    lseg = rel*2048 + ldst (relation-major), 128 blocks of 128 segs, 32
    psum-groups of 4 blocks (512 segs).
  - SPMD uniformity: per-(group,block) edge counts padded to the max over
    the 8 cores (cnt_ub), so the instruction stream (matmul lane runs,
    start/stop) is identical on every core while idx/slot tables are
    per-core data. Groups processed n4-major so the dense output GEMM for
    each 512-dst chunk can interleave with later aggregation.
  - One dma_gather per group (~2400 idxs) instead of many small batches:
    fewer fixed SWDGE overheads; trailing pads use idx=-1 which the Q7
    ucode trims (no descriptors, no DMA bytes). Mid-stream pads repeat the
    previous src row (HBM row-buffer hit). Edges sorted by src within each
    block for HBM locality of the random gather.
  - Aggregation per 128-edge tile: S[p,q] = (slot_p == q) built on DVE in
    batches of 8 tiles; tiles may straddle block boundaries - each
    (tile, block) lane run becomes its own matmul over the partition
    sub-range, accumulating mean^T in PSUM per group.
  - PSUM->SBUF mean copy fused with the 1/cnt multiply on DVE; final GEMM
    per (mc, n4): 16 relation + 2 root matmuls chained in one PSUM bank;
    bias applied by the Activation engine during copy-out; per-chunk output
    DMA overlaps the tail.
"""

import sys

if "/opt/trn_rl_repo" not in sys.path:
    sys.path.insert(0, "/opt/trn_rl_repo")

import numpy as np

B, L, H, R = 16, 512, 256, 8
N = B * 2 * L          # 16384 nodes
E = 524288
NCORES = 8
NPC = N // NCORES      # 2048 nodes per core
SEGS = NPC * R         # 16384 segments per core
NBLK = SEGS // 128     # 128 blocks per core
NGRP = NBLK // 4       # 32 psum groups per core
P = 128
GBUFS = 3              # gather tile pool depth

# group processing order: n4-major so the final GEMM for dst-chunk n4 can
# fire after positions 8*n4 .. 8*n4+7
PORDER = [r * 4 + n4 for n4 in range(4) for r in range(8)]

_COMPILED = {}         # meta_key -> (nc, meta)


def _plan(cnt_ub):
    """Uniform per-core plan from cnt_ub[128] (cross-core max per block).

    All matmuls are full 128-partition; a tile straddling a block boundary
    gets a second matmul driven by a masked slot table (s8B). Returns:
      nt[p]        tiles in group (processing order p)
      tiles[p]     per tile: (bposA, startA, stopA, bidx_or_-1, bposB,
                              startB, stopB) - bidx indexes the compact
                              boundary-tile slot table
      block_ofs[p] per block: stream offset within group
      bnd[p]       per tile: boundary lane (128 if none)
      nbt          number of boundary tiles
    """
    nt, tiles_all, blk_ofs, bnd_all = [], [], [], []
    nbt = 0
    for p, gid in enumerate(PORDER):
        cnts = [int(cnt_ub[4 * gid + i]) for i in range(4)]
        ofs = np.concatenate([[0], np.cumsum(cnts)])
        total = int(ofs[-1])
        ntg = (total + P - 1) // P
        padded = ntg * P
        # block intervals; last block absorbs tail pads (slot=-1 zeroes them)
        iv = [(int(ofs[i]), int(ofs[i + 1])) for i in range(4)]
        iv[3] = (iv[3][0], padded)
        mms = []          # emission order: (t, 'A'|'B', bpos)
        tmeta = []
        bnds = []
        for t in range(ntg):
            w0, w1 = t * P, (t + 1) * P
            hit = [(bpos, max(a, w0), min(b, w1))
                   for bpos, (a, b) in enumerate(iv)
                   if max(a, w0) < min(b, w1)]
            assert 1 <= len(hit) <= 2, f"tile spans {len(hit)} blocks"
            bposA = hit[0][0]
            mms.append((t, 'A', bposA))
            if len(hit) == 2:
                bposB = hit[1][0]
                bnds.append(hit[1][1] - w0)
                mms.append((t, 'B', bposB))
                tmeta.append([bposA, nbt, bposB])
                nbt += 1
            else:
                bnds.append(P)
                tmeta.append([bposA, -1, -1])
        first, last = {}, {}
        for i, (t, ab, bpos) in enumerate(mms):
            if bpos not in first:
                first[bpos] = (t, ab)
            last[bpos] = (t, ab)
        out_tiles = []
        for t in range(ntg):
            bposA, bidx, bposB = tmeta[t]
            stA = first[bposA] == (t, 'A')
            spA = last[bposA] == (t, 'A')
            if bidx >= 0:
                stB = first[bposB] == (t, 'B')
                spB = last[bposB] == (t, 'B')
            else:
                stB = spB = False
            out_tiles.append((bposA, stA, spA, bidx, bposB, stB, spB))
        nt.append(ntg)
        tiles_all.append(out_tiles)
        blk_ofs.append([int(o) for o in ofs[:4]])
        bnd_all.append(bnds)
    return {"nt": nt, "tiles": tiles_all, "block_ofs": blk_ofs,
            "bnd": bnd_all, "nbt": nbt}


def _meta_key(plan):
    return (tuple(plan["nt"]),
            tuple(tuple(tr) for g in plan["tiles"] for tr in g))


def _build_program(plan):
    """Build + compile the 8-core SPMD Bass program for this plan."""
    from concourse import bass, bacc, tile, mybir
    from concourse import library_config

    f32 = mybir.dt.float32
    f16 = mybir.dt.float16
    i16 = mybir.dt.int16
    nt = plan["nt"]
    tiles = plan["tiles"]
    NBT = max(plan["nbt"], 1)
    NT = sum(nt)                   # total tiles
    tile_ofs = np.concatenate([[0], np.cumsum(nt)]).astype(int)

    nc = bacc.Bacc("TRN2", target_bir_lowering=False, debug=False,
                   num_devices=NCORES, num_swdge_queues=4)

    i32 = mybir.dt.int32
    xtab = nc.dram_tensor("xtab", [N, H], f16, kind="ExternalInput")
    idxsd = nc.dram_tensor("idxsd", [P, NT * 8], i16, kind="ExternalInput")
    slotsd = nc.dram_tensor("slotsd", [P, NT], f16, kind="ExternalInput")
    slotsbd = nc.dram_tensor("slotsbd", [P, NBT], f16, kind="ExternalInput")
    cntd = nc.dram_tensor("cntd", [1, NGRP], i32, kind="ExternalInput")
    recipd = nc.dram_tensor("recipd", [P, SEGS], f16, kind="ExternalInput")
    iotad = nc.dram_tensor("iotad", [P, 8 * P], f16, kind="ExternalInput")
    wt = nc.dram_tensor("wt", [P, R * 2 * 2 * P], f16, kind="ExternalInput")
    roott = nc.dram_tensor("roott", [P, 2 * 2 * P], f16, kind="ExternalInput")
    biast = nc.dram_tensor("biast", [P, 2], f32, kind="ExternalInput")
    xt = nc.dram_tensor("xt", [P, 2 * NPC], f16, kind="ExternalInput")
    out = nc.dram_tensor("out", [2, P, NPC], f32, kind="ExternalOutput")

    with tile.TileContext(nc) as tc:
        with (
            tc.tile_pool(name="const", bufs=1) as cpool,
            tc.tile_pool(name="g", bufs=GBUFS) as gpool,
            tc.tile_pool(name="s", bufs=4) as spool,
            tc.tile_pool(name="pt", bufs=2, space="PSUM") as pt_pool,
            tc.tile_pool(name="po", bufs=2, space="PSUM") as po_pool,
        ):
            # critical loads first: gathers depend on idx table
            idx_sb = cpool.tile([P, NT * 8], i16)
            nc.sync.dma_start(idx_sb[:], idxsd.ap())
            slots_sb = cpool.tile([P, NT, 1], f16)
            nc.sync.dma_start(slots_sb[:], slotsd.ap())
            slotsb_sb = cpool.tile([P, NBT, 1], f16)
            nc.sync.dma_start(slotsb_sb[:], slotsbd.ap())
            iota_sb = cpool.tile([P, 8, P], f16)
            nc.sync.dma_start(iota_sb[:], iotad.ap())
            recip_sb = cpool.tile([P, SEGS], f16)
            nc.sync.dma_start(recip_sb[:], recipd.ap())
            w_sb = cpool.tile([P, R * 2 * 2 * P], f16)
            nc.sync.dma_start(w_sb[:], wt.ap())
            root_sb = cpool.tile([P, 2 * 2 * P], f16)
            nc.sync.dma_start(root_sb[:], roott.ap())
            bias_sb = cpool.tile([P, 2], f32)
            nc.sync.dma_start(bias_sb[:], biast.ap())
            xt_sb = cpool.tile([P, 2, NPC], f16)
            nc.sync.dma_start(xt_sb[:], xt.ap())

            mt = cpool.tile([P, 2, SEGS], f16)      # mean^T, all relations
            out_sb = cpool.tile([P, 2, NPC], f32)

            nc.gpsimd.load_library(library_config.mlp)

            g_tiles = {}
            s8_bufs = {}
            s8b_bufs = {}

            def ensure_s8(tt):
                """Build the S one-hot batch containing global tile tt."""
                b = tt // 8
                if b in s8_bufs:
                    return
                n = min(8, NT - b * 8)
                s8 = spool.tile([P, n, P], f16, name=f"s8_{b}", tag="s8")
                nc.vector.tensor_tensor(
                    out=s8[:], in0=iota_sb[:, :n, :],
                    in1=slots_sb[:, b * 8:b * 8 + n, :]
                    .to_broadcast([P, n, P]),
                    op=mybir.AluOpType.is_equal)
                s8_bufs[b] = s8

            def ensure_s8b(bidx):
                b = bidx // 8
                if b in s8b_bufs:
                    return
                n = min(8, plan["nbt"] - b * 8)
                s8b = spool.tile([P, n, P], f16, name=f"s8b_{b}", tag="s8b")
                nc.vector.tensor_tensor(
                    out=s8b[:], in0=iota_sb[:, :n, :],
                    in1=slotsb_sb[:, b * 8:b * 8 + n, :]
                    .to_broadcast([P, n, P]),
                    op=mybir.AluOpType.is_equal)
                s8b_bufs[b] = s8b

            for p, gid in enumerate(PORDER):
                ntg = nt[p]
                g = gpool.tile([P, ntg, H], f16, name=f"g{p}", tag="g")
                # <=1024 idxs per gather: larger batches overflow the SWDGE
                # descriptor-ring carveout and hang the Q7 (HW-only limit)
                for t0 in range(0, ntg, 8):
                    n8 = min(8, ntg - t0)
                    c0 = (int(tile_ofs[p]) + t0) * 8
                    nc.gpsimd.dma_gather(
                        g[:, t0:t0 + n8, :], xtab.ap(),
                        idx_sb[:, c0:c0 + n8 * 8],
                        num_idxs=n8 * P, num_idxs_reg=n8 * P, elem_size=H,
                        queue_num=p % 4)
                g_tiles[p] = g

                psT = [pt_pool.tile([P, 512], f32, name=f"psT{p}_{kc}",
                                    tag=f"psT{kc}")
                       for kc in range(2)]
                for t in range(ntg):
                    tt = int(tile_ofs[p]) + t
                    ensure_s8(tt)
                    s8 = s8_bufs[tt // 8]
                    ti = tt % 8
                    bposA, stA, spA, bidx, bposB, stB, spB = tiles[p][t]
                    for kc in range(2):
                        nc.tensor.matmul(
                            out=psT[kc][:, bposA * P:(bposA + 1) * P],
                            lhsT=g[:, t, kc * P:(kc + 1) * P],
                            rhs=s8[:, ti, :],
                            start=stA, stop=spA)
                    if bidx >= 0:
                        ensure_s8b(bidx)
                        s8b = s8b_bufs[bidx // 8]
                        for kc in range(2):
                            nc.tensor.matmul(
                                out=psT[kc][:, bposB * P:(bposB + 1) * P],
                                lhsT=g[:, t, kc * P:(kc + 1) * P],
                                rhs=s8b[:, bidx % 8, :],
                                start=stB, stop=spB)
                # mean = sum * recip, fused into the PSUM->SBUF copy
                for kc in range(2):
                    nc.vector.tensor_tensor(
                        out=mt[:, kc, gid * 512:(gid + 1) * 512],
                        in0=psT[kc][:],
                        in1=recip_sb[:, gid * 512:(gid + 1) * 512],
                        op=mybir.AluOpType.mult)

                if p % 8 == 7:
                    # all groups of dst-chunk n4 are aggregated: dense GEMM
                    n4 = p // 8
                    for mc in range(2):
                        po = po_pool.tile([P, 512], f32, name=f"po{n4}_{mc}",
                                          tag="po")
                        for r in range(R):
                            for kc in range(2):
                                wofs = ((r * 2 + kc) * 2 + mc) * P
                                nc.tensor.matmul(
                                    out=po[:],
                                    lhsT=w_sb[:, wofs:wofs + P],
                                    rhs=mt[:, kc, r * NPC + n4 * 512:
                                           r * NPC + (n4 + 1) * 512],
                                    start=(r == 0 and kc == 0), stop=False)
                        for kc in range(2):
                            rofs = (kc * 2 + mc) * P
                            nc.tensor.matmul(
                                out=po[:],
                                lhsT=root_sb[:, rofs:rofs + P],
                                rhs=xt_sb[:, kc, n4 * 512:(n4 + 1) * 512],
                                start=False, stop=(kc == 1))
                        nc.scalar.add(
                            out=out_sb[:, mc, n4 * 512:(n4 + 1) * 512],
                            in_=po[:], add=bias_sb[:, mc:mc + 1])
                        nc.sync.dma_start(
                            out.ap()[mc][:, n4 * 512:(n4 + 1) * 512],
                            out_sb[:, mc, n4 * 512:(n4 + 1) * 512])

    nc.compile()
    return nc


def _prep_inputs(input_s, input_a, edge_index, edge_type, weight, root, bias):
    """Host-side sharding/layout prep. Returns (plan, in_maps)."""
    x = np.ascontiguousarray(
        np.concatenate([input_s, input_a], axis=1).reshape(N, H)
    ).astype(np.float32)
    xtab = x.astype(np.float16)

    src = np.asarray(edge_index[0]).astype(np.int64)
    dst = np.asarray(edge_index[1]).astype(np.int64)
    et = np.asarray(edge_type).astype(np.int64)

    cnt = np.bincount(dst * R + et, minlength=N * R).reshape(N, R)
    recip_full = (1.0 / np.maximum(cnt, 1)).astype(np.float32)  # [N, R]

    owner = dst // NPC
    ldst = dst - owner * NPC
    lseg = et * NPC + ldst                          # relation-major local seg
    block = lseg >> 7                               # 0..127
    gid = block >> 2
    gkey = np.empty(NGRP, np.int64)                 # gid -> processing pos
    for pos, g in enumerate(PORDER):
        gkey[g] = pos

    cnt_cb = np.bincount(owner * NBLK + block,
                         minlength=NCORES * NBLK).reshape(NCORES, NBLK)
    cnt_ub = cnt_cb.max(axis=0)                     # [128]
    plan = _plan(cnt_ub)

    # stream layout (uniform across cores): per processing position p,
    # blocks 4*gid..4*gid+3 at offsets block_ofs, padded to nt[p]*128
    nt = plan["nt"]
    tile_ofs = np.concatenate([[0], np.cumsum(nt)]).astype(int)
    NT = int(tile_ofs[-1])
    stream_len = NT * P
    # absolute stream offset of each block (per core-uniform)
    blk_abs = np.zeros(NBLK, np.int64)
    grp_pad_end = np.zeros(NGRP, np.int64)          # padded end per position
    for p, g in enumerate(PORDER):
        base = int(tile_ofs[p]) * P
        for i in range(4):
            blk_abs[4 * g + i] = base + plan["block_ofs"][p][i]
        grp_pad_end[p] = int(tile_ofs[p + 1]) * P

    # sort edges by (owner, block, src); relies on blocks being laid out in
    # processing order via blk_abs when scattering below
    order = np.lexsort((src, block, owner))
    s_src = src[order].astype(np.int16)
    s_slot = (lseg[order] & 127).astype(np.float16)
    s_owner = owner[order]
    s_block = block[order]
    # position within (owner, block) run
    key = s_owner * NBLK + s_block
    starts = np.concatenate([[0], np.cumsum(np.bincount(
        key, minlength=NCORES * NBLK))])
    pos_in_blk = np.arange(E) - starts[key]

    idxs = np.full((NCORES, stream_len), -1, np.int16)
    slots = np.full((NCORES, stream_len), -1.0, np.float16)
    dest = blk_abs[s_block] + pos_in_blk
    idxs[s_owner, dest] = s_src
    slots[s_owner, dest] = s_slot

    # boundary tiles: mask the second block's lanes out of the main slot
    # table (A) and move them into the compact B table
    nbt = plan["nbt"]
    slotsB = np.full((NCORES, max(nbt, 1), P), -1.0, np.float16)
    for p in range(NGRP):
        for t in range(nt[p]):
            bidx = plan["tiles"][p][t][3]
            if bidx < 0:
                continue
            bl = plan["bnd"][p][t]
            base = (int(tile_ofs[p]) + t) * P
            slotsB[:, bidx, bl:] = slots[:, base + bl:base + P]
            slots[:, base + bl:base + P] = -1.0

    # mid-stream pads must be valid HBM rows: forward-fill idx within each
    # core's stream, but keep trailing -1 (trimmed by the ucode) for group
    # tails of positions >= GBUFS (earlier positions first-touch the pool
    # buffers, so fill those too to avoid stale NaN garbage)
    # a group may use idx=-1 trailing pads (trimmed by the ucode: no
    # descriptors, no DMA) only if every lane it leaves unwritten was
    # already written by an earlier group in the same pool slot; otherwise
    # fill its pads so the whole extent is gathered (finite)
    # (pool slot assignment is scheduler-controlled, so the only safe
    # policy is to write every lane of every group)
    fill_full = np.ones(NGRP, bool)

    nvalid = np.zeros((NCORES, NGRP), np.int32)
    for c in range(NCORES):
        row = idxs[c].astype(np.int32)
        invalid = row < 0
        ff = np.where(invalid, 0, np.arange(stream_len))
        np.maximum.accumulate(ff, out=ff)
        filled = row[ff]
        keep_neg = np.zeros(stream_len, bool)
        for p in range(NGRP):
            end = int(grp_pad_end[p])
            beg = int(tile_ofs[p]) * P
            if fill_full[p]:
                nvalid[c, p] = end - beg
                continue
            # trailing invalid run of this group's stream segment
            seg = invalid[beg:end]
            nz = np.flatnonzero(~seg)
            tail_start = (beg + nz[-1] + 1) if len(nz) else beg
            keep_neg[tail_start:end] = True
            nvalid[c, p] = tail_start - beg
        row = np.where(invalid & ~keep_neg, filled, row)
        row[invalid & keep_neg] = -1
        idxs[c] = row.astype(np.int16)

    iota_host = np.tile(np.arange(P, dtype=np.float16), (P, 8, 1)
                        ).reshape(P, 8 * P)

    w_host = np.ascontiguousarray(
        np.asarray(weight, np.float32).reshape(R, 2, P, 2, P)
        .transpose(2, 0, 1, 3, 4).reshape(P, R * 2 * 2 * P)).astype(np.float16)
    root_host = np.ascontiguousarray(
        np.asarray(root, np.float32).reshape(2, P, 2, P)
        .transpose(1, 0, 2, 3).reshape(P, 2 * 2 * P)).astype(np.float16)
    bias_host = np.ascontiguousarray(
        np.asarray(bias, np.float32).reshape(2, P).T)

    in_maps = []
    for c in range(NCORES):
        xc = x[c * NPC:(c + 1) * NPC]              # [2048, 256]
        xt_host = np.ascontiguousarray(
            xc.T.reshape(2, P, NPC).transpose(1, 0, 2).reshape(P, 2 * NPC)
        ).astype(np.float16)
        idx_host = np.ascontiguousarray(
            np.tile(idxs[c].reshape(NT * 8, 16).T, (8, 1)))
        rc = recip_full[c * NPC:(c + 1) * NPC, :].T.reshape(SEGS)
        recip_host = np.ascontiguousarray(
            np.broadcast_to(rc.astype(np.float16), (P, SEGS)))
        in_maps.append({
            "xtab": xtab,
            "idxsd": idx_host,
            "slotsd": np.ascontiguousarray(slots[c].reshape(NT, P).T),
            "slotsbd": np.ascontiguousarray(slotsB[c].T),
            "cntd": np.ascontiguousarray(nvalid[c].reshape(1, NGRP)),
            "recipd": recip_host,
            "iotad": iota_host,
            "wt": w_host,
            "roott": root_host,
            "biast": bias_host,
            "xt": xt_host,
        })
    return plan, in_maps


def _run(in_maps, plan, trace=False, trace_cores=None):
    from concourse import bass_utils
    key = _meta_key(plan)
    if key not in _COMPILED:
        _COMPILED[key] = _build_program(plan)
    nc = _COMPILED[key]
    kwargs = {}
    if trace:
        _install_ntff_shim()
        bass_utils.upload_artifacts = lambda tmpdir: tmpdir
        kwargs = dict(trace=True,
                      trace_cores=trace_cores if trace_cores else [0])
    return bass_utils.run_bass_kernel_spmd(
        nc, in_maps, core_ids=list(range(NCORES)), **kwargs)


def _assemble(results):
    full = np.empty((N, H), np.float32)
    for c in range(NCORES):
        o = results[c]["out"]                      # [2, 128, 2048]
        full[c * NPC:(c + 1) * NPC, 0:P] = o[0].T
        full[c * NPC:(c + 1) * NPC, P:2 * P] = o[1].T
    dtrp = full.reshape(B, 2 * L, H)
    sent = np.ascontiguousarray(dtrp[:, :L, :])
    act = np.ascontiguousarray(dtrp[:, L:, :])
    return sent, act


def kernel(input_s, input_a, edge_index, edge_type, weight, root, bias,
           _trace=False, _trace_cores=None, _return_stats=False):
    plan, in_maps = _prep_inputs(input_s, input_a, edge_index, edge_type,
                                 weight, root, bias)
    res = _run(in_maps, plan, trace=_trace, trace_cores=_trace_cores)
    out = _assemble(res.results)
    if _return_stats:
        return out, res
    return out


def _install_ntff_shim():
    """Install antenv.axon_hooks NTFF profiling hook via ctypes (the agent
    image lacks the module; same mechanism trn_boot would use)."""
    import types, ctypes, contextlib
    if "antenv.axon_hooks" in sys.modules:
        return
    so_path = "/opt/axon/libaxon_pjrt.so"
    lib = ctypes.CDLL(so_path)
    if not hasattr(lib, "axon_start_nrt_profile"):
        return
    lib.axon_start_nrt_profile.argtypes = [ctypes.POINTER(ctypes.c_int64),
                                           ctypes.c_size_t]
    lib.axon_start_nrt_profile.restype = ctypes.c_int64
    lib.axon_stop_nrt_profile.argtypes = [ctypes.c_char_p]
    lib.axon_stop_nrt_profile.restype = ctypes.c_int64

    @contextlib.contextmanager
    def _hook(output_dir, device_ids):
        import jax
        jax.devices()
        if device_ids:
            ids = (ctypes.c_int64 * len(device_ids))(*device_ids)
            rc = lib.axon_start_nrt_profile(ids, len(device_ids))
        else:
            rc = lib.axon_start_nrt_profile(None, 0)
        if rc != 0:
            raise RuntimeError(f"axon_start_nrt_profile rc={rc}")
        try:
            yield
        finally:
            n = lib.axon_stop_nrt_profile(str(output_dir).encode())
            if n < 0:
                raise RuntimeError(f"axon_stop_nrt_profile rc={n}")

    import antenv
    mod = types.ModuleType("antenv.axon_hooks")
    mod.get_axon_ntff_profile_hook = lambda: _hook
    mod.set_axon_ntff_profile_hook = lambda h: None
    sys.modules["antenv.axon_hooks"] = mod
    antenv.axon_hooks = mod


# revision 23
# speedup vs baseline: 1.2573x; 1.2573x over previous
"""RGCN (mean-aggregation) message-passing kernel for 8 Trainium2 NeuronCores.

Problem shapes (hardcoded):
  B=16, L=512, H=256, R=8, E=524288, N = B*2*L = 16384 nodes.

Strategy v3 (dst-sharded, no collectives):
  - Node features x = concat(input_s, input_a) -> [N, H] fp16 table in HBM,
    replicated per core. Core c owns dst nodes [2048c, 2048(c+1)); segments
    lseg = rel*2048 + ldst (relation-major), 128 blocks of 128 segs, 32
    psum-groups of 4 blocks (512 segs).
  - SPMD uniformity: per-(group,block) edge counts padded to the max over
    the 8 cores (cnt_ub), so the instruction stream (matmul lane runs,
    start/stop) is identical on every core while idx/slot tables are
    per-core data. Groups processed n4-major so the dense output GEMM for
    each 512-dst chunk can interleave with later aggregation.
  - One dma_gather per group (~2400 idxs) instead of many small batches:
    fewer fixed SWDGE overheads; trailing pads use idx=-1 which the Q7
    ucode trims (no descriptors, no DMA bytes). Mid-stream pads repeat the
    previous src row (HBM row-buffer hit). Edges sorted by src within each
    block for HBM locality of the random gather.
  - Aggregation per 128-edge tile: S[p,q] = (slot_p == q) built on DVE in
    batches of 8 tiles; tiles may straddle block boundaries - each
    (tile, block) lane run becomes its own matmul over the partition
    sub-range, accumulating mean^T in PSUM per group.
  - PSUM->SBUF mean copy fused with the 1/cnt multiply on DVE; final GEMM
    per (mc, n4): 16 relation + 2 root matmuls chained in one PSUM bank;
    bias applied by the Activation engine during copy-out; per-chunk output
    DMA overlaps the tail.
"""

import sys

if "/opt/trn_rl_repo" not in sys.path:
    sys.path.insert(0, "/opt/trn_rl_repo")

import numpy as np

B, L, H, R = 16, 512, 256, 8
N = B * 2 * L          # 16384 nodes
E = 524288
NCORES = 8
NPC = N // NCORES      # 2048 nodes per core
SEGS = NPC * R         # 16384 segments per core
NBLK = SEGS // 128     # 128 blocks per core
NGRP = NBLK // 4       # 32 psum groups per core
P = 128
GBUFS = 3              # gather tile pool depth
GATHER_TILES = 8       # tiles per dma_gather instruction (1024 idxs; >=1536 hangs HW)

# group processing order: n4-major so the final GEMM for dst-chunk n4 can
# fire after positions 8*n4 .. 8*n4+7
PORDER = [r * 4 + n4 for n4 in range(4) for r in range(8)]

_COMPILED = {}         # meta_key -> (nc, meta)


def _plan(cnt_ub):
    """Uniform per-core plan from cnt_ub[128] (cross-core max per block).

    All matmuls are full 128-partition; a tile straddling a block boundary
    gets a second matmul driven by a masked slot table (s8B). Returns:
      nt[p]        tiles in group (processing order p)
      tiles[p]     per tile: (bposA, startA, stopA, bidx_or_-1, bposB,
                              startB, stopB) - bidx indexes the compact
                              boundary-tile slot table
      block_ofs[p] per block: stream offset within group
      bnd[p]       per tile: boundary lane (128 if none)
      nbt          number of boundary tiles
    """
    nt, tiles_all, blk_ofs, bnd_all = [], [], [], []
    nbt = 0
    for p, gid in enumerate(PORDER):
        cnts = [int(cnt_ub[4 * gid + i]) for i in range(4)]
        ofs = np.concatenate([[0], np.cumsum(cnts)])
        total = int(ofs[-1])
        ntg = (total + P - 1) // P
        padded = ntg * P
        # block intervals; last block absorbs tail pads (slot=-1 zeroes them)
        iv = [(int(ofs[i]), int(ofs[i + 1])) for i in range(4)]
        iv[3] = (iv[3][0], padded)
        mms = []          # emission order: (t, 'A'|'B', bpos)
        tmeta = []
        bnds = []
        for t in range(ntg):
            w0, w1 = t * P, (t + 1) * P
            hit = [(bpos, max(a, w0), min(b, w1))
                   for bpos, (a, b) in enumerate(iv)
                   if max(a, w0) < min(b, w1)]
            assert 1 <= len(hit) <= 2, f"tile spans {len(hit)} blocks"
            bposA = hit[0][0]
            mms.append((t, 'A', bposA))
            if len(hit) == 2:
                bposB = hit[1][0]
                bnds.append(hit[1][1] - w0)
                mms.append((t, 'B', bposB))
                tmeta.append([bposA, nbt, bposB])
                nbt += 1
            else:
                bnds.append(P)
                tmeta.append([bposA, -1, -1])
        first, last = {}, {}
        for i, (t, ab, bpos) in enumerate(mms):
            if bpos not in first:
                first[bpos] = (t, ab)
            last[bpos] = (t, ab)
        out_tiles = []
        for t in range(ntg):
            bposA, bidx, bposB = tmeta[t]
            stA = first[bposA] == (t, 'A')
            spA = last[bposA] == (t, 'A')
            if bidx >= 0:
                stB = first[bposB] == (t, 'B')
                spB = last[bposB] == (t, 'B')
            else:
                stB = spB = False
            out_tiles.append((bposA, stA, spA, bidx, bposB, stB, spB))
        nt.append(ntg)
        tiles_all.append(out_tiles)
        blk_ofs.append([int(o) for o in ofs[:4]])
        bnd_all.append(bnds)
    return {"nt": nt, "tiles": tiles_all, "block_ofs": blk_ofs,
            "bnd": bnd_all, "nbt": nbt}


def _meta_key(plan):
    return (tuple(plan["nt"]),
            tuple(tuple(tr) for g in plan["tiles"] for tr in g))


def _build_program(plan):
    """Build + compile the 8-core SPMD Bass program for this plan."""
    from concourse import bass, bacc, tile, mybir
    from concourse import library_config

    f32 = mybir.dt.float32
    f16 = mybir.dt.float16
    i16 = mybir.dt.int16
    nt = plan["nt"]
    tiles = plan["tiles"]
    NBT = max(plan["nbt"], 1)
    NT = sum(nt)                   # total tiles
    tile_ofs = np.concatenate([[0], np.cumsum(nt)]).astype(int)

    nc = bacc.Bacc("TRN2", target_bir_lowering=False, debug=False,
                   num_devices=NCORES, num_swdge_queues=4)

    i32 = mybir.dt.int32
    xtab = nc.dram_tensor("xtab", [N, H], f16, kind="ExternalInput")
    idxsd = nc.dram_tensor("idxsd", [P, NT * 8], i16, kind="ExternalInput")
    slotsd = nc.dram_tensor("slotsd", [P, NT], f16, kind="ExternalInput")
    slotsbd = nc.dram_tensor("slotsbd", [P, NBT], f16, kind="ExternalInput")
    cntd = nc.dram_tensor("cntd", [1, NGRP], i32, kind="ExternalInput")
    recipd = nc.dram_tensor("recipd", [P, SEGS], f16, kind="ExternalInput")
    iotad = nc.dram_tensor("iotad", [P, 8 * P], f16, kind="ExternalInput")
    wt = nc.dram_tensor("wt", [P, R * 2 * 2 * P], f16, kind="ExternalInput")
    roott = nc.dram_tensor("roott", [P, 2 * 2 * P], f16, kind="ExternalInput")
    biast = nc.dram_tensor("biast", [P, 2], f32, kind="ExternalInput")
    xt = nc.dram_tensor("xt", [P, 2 * NPC], f16, kind="ExternalInput")
    out = nc.dram_tensor("out", [2, P, NPC], f32, kind="ExternalOutput")

    with tile.TileContext(nc) as tc:
        with (
            tc.tile_pool(name="const", bufs=1) as cpool,
            tc.tile_pool(name="g", bufs=GBUFS) as gpool,
            tc.tile_pool(name="s", bufs=4) as spool,
            tc.tile_pool(name="pt", bufs=2, space="PSUM") as pt_pool,
            tc.tile_pool(name="po", bufs=2, space="PSUM") as po_pool,
        ):
            # critical loads first: gathers depend on idx table; split the
            # load so set-0 gathers start as soon as their slice lands
            NT0 = int(tile_ofs[8])
            idx_sb = cpool.tile([P, NT * 8], i16)
            nc.sync.dma_start(idx_sb[:, :NT0 * 8], idxsd.ap()[:, :NT0 * 8])
            nc.sync.dma_start(idx_sb[:, NT0 * 8:], idxsd.ap()[:, NT0 * 8:])
            slots_sb = cpool.tile([P, NT, 1], f16)
            nc.sync.dma_start(slots_sb[:], slotsd.ap())
            slotsb_sb = cpool.tile([P, NBT, 1], f16)
            nc.sync.dma_start(slotsb_sb[:], slotsbd.ap())
            iota_sb = cpool.tile([P, 8, P], f16)
            nc.sync.dma_start(iota_sb[:], iotad.ap())
            recip_sb = cpool.tile([P, SEGS], f16)
            nc.sync.dma_start(recip_sb[:], recipd.ap())
            w_sb = cpool.tile([P, R * 2 * 2 * P], f16)
            nc.sync.dma_start(w_sb[:], wt.ap())
            root_sb = cpool.tile([P, 2 * 2 * P], f16)
            nc.sync.dma_start(root_sb[:], roott.ap())
            bias_sb = cpool.tile([P, 2], f32)
            nc.sync.dma_start(bias_sb[:], biast.ap())
            xt_sb = cpool.tile([P, 2, NPC], f16)
            nc.sync.dma_start(xt_sb[:], xt.ap())

            mt = cpool.tile([P, 2, SEGS], f16)      # mean^T, all relations
            out_sb = cpool.tile([P, 2, NPC], f32)

            nc.gpsimd.load_library(library_config.mlp)

            g_tiles = {}
            s8_bufs = {}
            s8b_bufs = {}
            ndma = [0]

            def ensure_s8(tt):
                """Build the S one-hot batch containing global tile tt."""
                b = tt // 8
                if b in s8_bufs:
                    return
                n = min(8, NT - b * 8)
                s8 = spool.tile([P, n, P], f16, name=f"s8_{b}", tag="s8")
                nc.vector.tensor_tensor(
                    out=s8[:], in0=iota_sb[:, :n, :],
                    in1=slots_sb[:, b * 8:b * 8 + n, :]
                    .to_broadcast([P, n, P]),
                    op=mybir.AluOpType.is_equal)
                s8_bufs[b] = s8

            def ensure_s8b(bidx):
                b = bidx // 8
                if b in s8b_bufs:
                    return
                n = min(8, plan["nbt"] - b * 8)
                s8b = spool.tile([P, n, P], f16, name=f"s8b_{b}", tag="s8b")
                nc.vector.tensor_tensor(
                    out=s8b[:], in0=iota_sb[:, :n, :],
                    in1=slotsb_sb[:, b * 8:b * 8 + n, :]
                    .to_broadcast([P, n, P]),
                    op=mybir.AluOpType.is_equal)
                s8b_bufs[b] = s8b

            for p, gid in enumerate(PORDER):
                ntg = nt[p]
                g = gpool.tile([P, ntg, H], f16, name=f"g{p}", tag="g")
                # queue must follow the tile scheduler's global DMASW lane
                # rotation (lane = counter % 8, queue = lane % 4) so each
                # completion sem stays on one queue
                for t0 in range(0, ntg, GATHER_TILES):
                    n8 = min(GATHER_TILES, ntg - t0)
                    c0 = (int(tile_ofs[p]) + t0) * 8
                    nc.gpsimd.dma_gather(
                        g[:, t0:t0 + n8, :], xtab.ap(),
                        idx_sb[:, c0:c0 + n8 * 8],
                        num_idxs=n8 * P, num_idxs_reg=n8 * P, elem_size=H,
                        queue_num=ndma[0] % 4)
                    ndma[0] += 1
                g_tiles[p] = g

                psT = [pt_pool.tile([P, 512], f32, name=f"psT{p}_{kc}",
                                    tag=f"psT{kc}")
                       for kc in range(2)]
                for t in range(ntg):
                    tt = int(tile_ofs[p]) + t
                    ensure_s8(tt)
                    s8 = s8_bufs[tt // 8]
                    ti = tt % 8
                    bposA, stA, spA, bidx, bposB, stB, spB = tiles[p][t]
                    for kc in range(2):
                        nc.tensor.matmul(
                            out=psT[kc][:, bposA * P:(bposA + 1) * P],
                            lhsT=g[:, t, kc * P:(kc + 1) * P],
                            rhs=s8[:, ti, :],
                            start=stA, stop=spA)
                    if bidx >= 0:
                        ensure_s8b(bidx)
                        s8b = s8b_bufs[bidx // 8]
                        for kc in range(2):
                            nc.tensor.matmul(
                                out=psT[kc][:, bposB * P:(bposB + 1) * P],
                                lhsT=g[:, t, kc * P:(kc + 1) * P],
                                rhs=s8b[:, bidx % 8, :],
                                start=stB, stop=spB)
                # mean = sum * recip, fused into the PSUM->SBUF copy
                for kc in range(2):
                    nc.vector.tensor_tensor(
                        out=mt[:, kc, gid * 512:(gid + 1) * 512],
                        in0=psT[kc][:],
                        in1=recip_sb[:, gid * 512:(gid + 1) * 512],
                        op=mybir.AluOpType.mult)

                if p % 8 == 7:
                    # all groups of dst-chunk n4 are aggregated: dense GEMM
                    n4 = p // 8
                    for mc in range(2):
                        po = po_pool.tile([P, 512], f32, name=f"po{n4}_{mc}",
                                          tag="po")
                        for r in range(R):
                            for kc in range(2):
                                wofs = ((r * 2 + kc) * 2 + mc) * P
                                nc.tensor.matmul(
                                    out=po[:],
                                    lhsT=w_sb[:, wofs:wofs + P],
                                    rhs=mt[:, kc, r * NPC + n4 * 512:
                                           r * NPC + (n4 + 1) * 512],
                                    start=(r == 0 and kc == 0), stop=False)
                        for kc in range(2):
                            rofs = (kc * 2 + mc) * P
                            nc.tensor.matmul(
                                out=po[:],
                                lhsT=root_sb[:, rofs:rofs + P],
                                rhs=xt_sb[:, kc, n4 * 512:(n4 + 1) * 512],
                                start=False, stop=(kc == 1))
                        nc.scalar.add(
                            out=out_sb[:, mc, n4 * 512:(n4 + 1) * 512],
                            in_=po[:], add=bias_sb[:, mc:mc + 1])
                        nc.sync.dma_start(
                            out.ap()[mc][:, n4 * 512:(n4 + 1) * 512],
                            out_sb[:, mc, n4 * 512:(n4 + 1) * 512])

    nc.compile()
    return nc


def _prep_inputs(input_s, input_a, edge_index, edge_type, weight, root, bias):
    """Host-side sharding/layout prep. Returns (plan, in_maps)."""
    x = np.ascontiguousarray(
        np.concatenate([input_s, input_a], axis=1).reshape(N, H)
    ).astype(np.float32)
    xtab = x.astype(np.float16)

    src = np.asarray(edge_index[0]).astype(np.int64)
    dst = np.asarray(edge_index[1]).astype(np.int64)
    et = np.asarray(edge_type).astype(np.int64)

    cnt = np.bincount(dst * R + et, minlength=N * R).reshape(N, R)
    recip_full = (1.0 / np.maximum(cnt, 1)).astype(np.float32)  # [N, R]

    owner = dst // NPC
    ldst = dst - owner * NPC
    lseg = et * NPC + ldst                          # relation-major local seg
    block = lseg >> 7                               # 0..127
    gid = block >> 2
    gkey = np.empty(NGRP, np.int64)                 # gid -> processing pos
    for pos, g in enumerate(PORDER):
        gkey[g] = pos

    cnt_cb = np.bincount(owner * NBLK + block,
                         minlength=NCORES * NBLK).reshape(NCORES, NBLK)
    cnt_ub = cnt_cb.max(axis=0)                     # [128]
    plan = _plan(cnt_ub)

    # stream layout (uniform across cores): per processing position p,
    # blocks 4*gid..4*gid+3 at offsets block_ofs, padded to nt[p]*128
    nt = plan["nt"]
    tile_ofs = np.concatenate([[0], np.cumsum(nt)]).astype(int)
    NT = int(tile_ofs[-1])
    stream_len = NT * P
    # absolute stream offset of each block (per core-uniform)
    blk_abs = np.zeros(NBLK, np.int64)
    grp_pad_end = np.zeros(NGRP, np.int64)          # padded end per position
    for p, g in enumerate(PORDER):
        base = int(tile_ofs[p]) * P
        for i in range(4):
            blk_abs[4 * g + i] = base + plan["block_ofs"][p][i]
        grp_pad_end[p] = int(tile_ofs[p + 1]) * P

    # sort edges by (owner, block, src); relies on blocks being laid out in
    # processing order via blk_abs when scattering below
    order = np.lexsort((src, block, owner))
    s_src = src[order].astype(np.int16)
    s_slot = (lseg[order] & 127).astype(np.float16)
    s_owner = owner[order]
    s_block = block[order]
    # position within (owner, block) run
    key = s_owner * NBLK + s_block
    starts = np.concatenate([[0], np.cumsum(np.bincount(
        key, minlength=NCORES * NBLK))])
    pos_in_blk = np.arange(E) - starts[key]

    idxs = np.full((NCORES, stream_len), -1, np.int16)
    slots = np.full((NCORES, stream_len), -1.0, np.float16)
    dest = blk_abs[s_block] + pos_in_blk
    idxs[s_owner, dest] = s_src
    slots[s_owner, dest] = s_slot

    # boundary tiles: mask the second block's lanes out of the main slot
    # table (A) and move them into the compact B table
    nbt = plan["nbt"]
    slotsB = np.full((NCORES, max(nbt, 1), P), -1.0, np.float16)
    for p in range(NGRP):
        for t in range(nt[p]):
            bidx = plan["tiles"][p][t][3]
            if bidx < 0:
                continue
            bl = plan["bnd"][p][t]
            base = (int(tile_ofs[p]) + t) * P
            slotsB[:, bidx, bl:] = slots[:, base + bl:base + P]
            slots[:, base + bl:base + P] = -1.0

    # mid-stream pads must be valid HBM rows: forward-fill idx within each
    # core's stream, but keep trailing -1 (trimmed by the ucode) for group
    # tails of positions >= GBUFS (earlier positions first-touch the pool
    # buffers, so fill those too to avoid stale NaN garbage)
    # a group may use idx=-1 trailing pads (trimmed by the ucode: no
    # descriptors, no DMA) only if every lane it leaves unwritten was
    # already written by an earlier group in the same pool slot; otherwise
    # fill its pads so the whole extent is gathered (finite)
    # (pool slot assignment is scheduler-controlled, so the only safe
    # policy is to write every lane of every group)
    fill_full = np.ones(NGRP, bool)

    nvalid = np.zeros((NCORES, NGRP), np.int32)
    for c in range(NCORES):
        row = idxs[c].astype(np.int32)
        invalid = row < 0
        ff = np.where(invalid, 0, np.arange(stream_len))
        np.maximum.accumulate(ff, out=ff)
        filled = row[ff]
        keep_neg = np.zeros(stream_len, bool)
        for p in range(NGRP):
            end = int(grp_pad_end[p])
            beg = int(tile_ofs[p]) * P
            if fill_full[p]:
                nvalid[c, p] = end - beg
                continue
            # trailing invalid run of this group's stream segment
            seg = invalid[beg:end]
            nz = np.flatnonzero(~seg)
            tail_start = (beg + nz[-1] + 1) if len(nz) else beg
            keep_neg[tail_start:end] = True
            nvalid[c, p] = tail_start - beg
        row = np.where(invalid & ~keep_neg, filled, row)
        row[invalid & keep_neg] = -1
        idxs[c] = row.astype(np.int16)

    iota_host = np.tile(np.arange(P, dtype=np.float16), (P, 8, 1)
                        ).reshape(P, 8 * P)

    w_host = np.ascontiguousarray(
        np.asarray(weight, np.float32).reshape(R, 2, P, 2, P)
        .transpose(2, 0, 1, 3, 4).reshape(P, R * 2 * 2 * P)).astype(np.float16)
    root_host = np.ascontiguousarray(
        np.asarray(root, np.float32).reshape(2, P, 2, P)
        .transpose(1, 0, 2, 3).reshape(P, 2 * 2 * P)).astype(np.float16)
    bias_host = np.ascontiguousarray(
        np.asarray(bias, np.float32).reshape(2, P).T)

    in_maps = []
    for c in range(NCORES):
        xc = x[c * NPC:(c + 1) * NPC]              # [2048, 256]
        xt_host = np.ascontiguousarray(
            xc.T.reshape(2, P, NPC).transpose(1, 0, 2).reshape(P, 2 * NPC)
        ).astype(np.float16)
        idx_host = np.ascontiguousarray(
            np.tile(idxs[c].reshape(NT * 8, 16).T, (8, 1)))
        rc = recip_full[c * NPC:(c + 1) * NPC, :].T.reshape(SEGS)
        recip_host = np.ascontiguousarray(
            np.broadcast_to(rc.astype(np.float16), (P, SEGS)))
        in_maps.append({
            "xtab": xtab,
            "idxsd": idx_host,
            "slotsd": np.ascontiguousarray(slots[c].reshape(NT, P).T),
            "slotsbd": np.ascontiguousarray(slotsB[c].T),
            "cntd": np.ascontiguousarray(nvalid[c].reshape(1, NGRP)),
            "recipd": recip_host,
            "iotad": iota_host,
            "wt": w_host,
            "roott": root_host,
            "biast": bias_host,
            "xt": xt_host,
        })
    return plan, in_maps


def _run(in_maps, plan, trace=False, trace_cores=None):
    from concourse import bass_utils
    key = _meta_key(plan)
    if key not in _COMPILED:
        _COMPILED[key] = _build_program(plan)
    nc = _COMPILED[key]
    kwargs = {}
    if trace:
        _install_ntff_shim()
        bass_utils.upload_artifacts = lambda tmpdir: tmpdir
        kwargs = dict(trace=True,
                      trace_cores=trace_cores if trace_cores else [0])
    return bass_utils.run_bass_kernel_spmd(
        nc, in_maps, core_ids=list(range(NCORES)), **kwargs)


def _assemble(results):
    full = np.empty((N, H), np.float32)
    for c in range(NCORES):
        o = results[c]["out"]                      # [2, 128, 2048]
        full[c * NPC:(c + 1) * NPC, 0:P] = o[0].T
        full[c * NPC:(c + 1) * NPC, P:2 * P] = o[1].T
    dtrp = full.reshape(B, 2 * L, H)
    sent = np.ascontiguousarray(dtrp[:, :L, :])
    act = np.ascontiguousarray(dtrp[:, L:, :])
    return sent, act


def kernel(input_s, input_a, edge_index, edge_type, weight, root, bias,
           _trace=False, _trace_cores=None, _return_stats=False):
    plan, in_maps = _prep_inputs(input_s, input_a, edge_index, edge_type,
                                 weight, root, bias)
    res = _run(in_maps, plan, trace=_trace, trace_cores=_trace_cores)
    out = _assemble(res.results)
    if _return_stats:
        return out, res
    return out


def _install_ntff_shim():
    """Install antenv.axon_hooks NTFF profiling hook via ctypes (the agent
    image lacks the module; same mechanism trn_boot would use)."""
    import types, ctypes, contextlib
    if "antenv.axon_hooks" in sys.modules:
        return
    so_path = "/opt/axon/libaxon_pjrt.so"
    lib = ctypes.CDLL(so_path)
    if not hasattr(lib, "axon_start_nrt_profile"):
        return
    lib.axon_start_nrt_profile.argtypes = [ctypes.POINTER(ctypes.c_int64),
                                           ctypes.c_size_t]
    lib.axon_start_nrt_profile.restype = ctypes.c_int64
    lib.axon_stop_nrt_profile.argtypes = [ctypes.c_char_p]
    lib.axon_stop_nrt_profile.restype = ctypes.c_int64

    @contextlib.contextmanager
    def _hook(output_dir, device_ids):
        import jax
        jax.devices()
        if device_ids:
            ids = (ctypes.c_int64 * len(device_ids))(*device_ids)
            rc = lib.axon_start_nrt_profile(ids, len(device_ids))
        else:
            rc = lib.axon_start_nrt_profile(None, 0)
        if rc != 0:
            raise RuntimeError(f"axon_start_nrt_profile rc={rc}")
        try:
            yield
        finally:
            n = lib.axon_stop_nrt_profile(str(output_dir).encode())
            if n < 0:
                raise RuntimeError(f"axon_stop_nrt_profile rc={n}")

    import antenv
    mod = types.ModuleType("antenv.axon_hooks")
    mod.get_axon_ntff_profile_hook = lambda: _hook
    mod.set_axon_ntff_profile_hook = lambda h: None
    sys.modules["antenv.axon_hooks"] = mod
    antenv.axon_hooks = mod


# revision 24
# speedup vs baseline: 1.2731x; 1.0126x over previous
"""RGCN (mean-aggregation) message-passing kernel for 8 Trainium2 NeuronCores.

Problem shapes (hardcoded):
  B=16, L=512, H=256, R=8, E=524288, N = B*2*L = 16384 nodes.

Strategy v3 (dst-sharded, no collectives):
  - Node features x = concat(input_s, input_a) -> [N, H] fp16 table in HBM,
    replicated per core. Core c owns dst nodes [2048c, 2048(c+1)); segments
    lseg = rel*2048 + ldst (relation-major), 128 blocks of 128 segs, 32
    psum-groups of 4 blocks (512 segs).
  - SPMD uniformity: per-(group,block) edge counts padded to the max over
    the 8 cores (cnt_ub), so the instruction stream (matmul lane runs,
    start/stop) is identical on every core while idx/slot tables are
    per-core data. Groups processed n4-major so the dense output GEMM for
    each 512-dst chunk can interleave with later aggregation.
  - dma_gather in 1024-idx batches (>=1536 idxs/instruction hangs the HW
    SWDGE ring); queue_num follows the tile scheduler's global DMASW lane
    rotation (queue = counter % 4) so each completion sem stays on one
    queue. All pads gather a repeated valid src row (slot=-1 zeroes their
    S contribution). Edges sorted by src within each block for HBM
    locality of the random gather (~2x effective bandwidth).
  - Aggregation per 128-edge tile: S[p,q] = (slot_p == q) built on DVE in
    batches of 8 tiles; a tile straddling a block boundary gets a second
    full-128-partition matmul driven by a compact masked slot table (s8B),
    avoiding PE tiling-mode switches; mean^T accumulates in PSUM per group.
  - PSUM->SBUF mean copy fused with the 1/cnt multiply on DVE; final GEMM
    per (mc, n4): 16 relation + 2 root matmuls chained in one PSUM bank;
    bias applied by the Activation engine during copy-out; per-chunk output
    DMA overlaps the tail.
"""

import sys

if "/opt/trn_rl_repo" not in sys.path:
    sys.path.insert(0, "/opt/trn_rl_repo")

import numpy as np

B, L, H, R = 16, 512, 256, 8
N = B * 2 * L          # 16384 nodes
E = 524288
NCORES = 8
NPC = N // NCORES      # 2048 nodes per core
SEGS = NPC * R         # 16384 segments per core
NBLK = SEGS // 128     # 128 blocks per core
NGRP = NBLK // 4       # 32 psum groups per core
P = 128
GBUFS = 3              # gather tile pool depth
GATHER_TILES = 8       # tiles per dma_gather instruction (1024 idxs; >=1536 hangs HW)

# group processing order: n4-major so the final GEMM for dst-chunk n4 can
# fire after positions 8*n4 .. 8*n4+7
PORDER = [r * 4 + n4 for n4 in range(4) for r in range(8)]

_COMPILED = {}         # meta_key -> (nc, meta)


def _plan(cnt_ub):
    """Uniform per-core plan from cnt_ub[128] (cross-core max per block).

    All matmuls are full 128-partition; a tile straddling a block boundary
    gets a second matmul driven by a masked slot table (s8B). Returns:
      nt[p]        tiles in group (processing order p)
      tiles[p]     per tile: (bposA, startA, stopA, bidx_or_-1, bposB,
                              startB, stopB) - bidx indexes the compact
                              boundary-tile slot table
      block_ofs[p] per block: stream offset within group
      bnd[p]       per tile: boundary lane (128 if none)
      nbt          number of boundary tiles
    """
    nt, tiles_all, blk_ofs, bnd_all = [], [], [], []
    nbt = 0
    for p, gid in enumerate(PORDER):
        cnts = [int(cnt_ub[4 * gid + i]) for i in range(4)]
        ofs = np.concatenate([[0], np.cumsum(cnts)])
        total = int(ofs[-1])
        ntg = (total + P - 1) // P
        padded = ntg * P
        # block intervals; last block absorbs tail pads (slot=-1 zeroes them)
        iv = [(int(ofs[i]), int(ofs[i + 1])) for i in range(4)]
        iv[3] = (iv[3][0], padded)
        mms = []          # emission order: (t, 'A'|'B', bpos)
        tmeta = []
        bnds = []
        for t in range(ntg):
            w0, w1 = t * P, (t + 1) * P
            hit = [(bpos, max(a, w0), min(b, w1))
                   for bpos, (a, b) in enumerate(iv)
                   if max(a, w0) < min(b, w1)]
            assert 1 <= len(hit) <= 2, f"tile spans {len(hit)} blocks"
            bposA = hit[0][0]
            mms.append((t, 'A', bposA))
            if len(hit) == 2:
                bposB = hit[1][0]
                bnds.append(hit[1][1] - w0)
                mms.append((t, 'B', bposB))
                tmeta.append([bposA, nbt, bposB])
                nbt += 1
            else:
                bnds.append(P)
                tmeta.append([bposA, -1, -1])
        first, last = {}, {}
        for i, (t, ab, bpos) in enumerate(mms):
            if bpos not in first:
                first[bpos] = (t, ab)
            last[bpos] = (t, ab)
        out_tiles = []
        for t in range(ntg):
            bposA, bidx, bposB = tmeta[t]
            stA = first[bposA] == (t, 'A')
            spA = last[bposA] == (t, 'A')
            if bidx >= 0:
                stB = first[bposB] == (t, 'B')
                spB = last[bposB] == (t, 'B')
            else:
                stB = spB = False
            out_tiles.append((bposA, stA, spA, bidx, bposB, stB, spB))
        nt.append(ntg)
        tiles_all.append(out_tiles)
        blk_ofs.append([int(o) for o in ofs[:4]])
        bnd_all.append(bnds)
    return {"nt": nt, "tiles": tiles_all, "block_ofs": blk_ofs,
            "bnd": bnd_all, "nbt": nbt}


def _meta_key(plan):
    return (tuple(plan["nt"]),
            tuple(tuple(tr) for g in plan["tiles"] for tr in g))


def _build_program(plan):
    """Build + compile the 8-core SPMD Bass program for this plan."""
    from concourse import bass, bacc, tile, mybir
    from concourse import library_config

    f32 = mybir.dt.float32
    f16 = mybir.dt.float16
    i16 = mybir.dt.int16
    nt = plan["nt"]
    tiles = plan["tiles"]
    NBT = max(plan["nbt"], 1)
    NT = sum(nt)                   # total tiles
    tile_ofs = np.concatenate([[0], np.cumsum(nt)]).astype(int)

    nc = bacc.Bacc("TRN2", target_bir_lowering=False, debug=False,
                   num_devices=NCORES, num_swdge_queues=4)

    i32 = mybir.dt.int32
    xtab = nc.dram_tensor("xtab", [N, H], f16, kind="ExternalInput")
    idxsd = nc.dram_tensor("idxsd", [P, NT * 8], i16, kind="ExternalInput")
    slotsd = nc.dram_tensor("slotsd", [P, NT], f16, kind="ExternalInput")
    slotsbd = nc.dram_tensor("slotsbd", [P, NBT], f16, kind="ExternalInput")
    cntd = nc.dram_tensor("cntd", [1, NGRP], i32, kind="ExternalInput")
    recipd = nc.dram_tensor("recipd", [P, SEGS], f16, kind="ExternalInput")
    iotad = nc.dram_tensor("iotad", [P, 8 * P], f16, kind="ExternalInput")
    wt = nc.dram_tensor("wt", [P, R * 2 * 2 * P], f16, kind="ExternalInput")
    roott = nc.dram_tensor("roott", [P, 2 * 2 * P], f16, kind="ExternalInput")
    biast = nc.dram_tensor("biast", [P, 2], f32, kind="ExternalInput")
    xt = nc.dram_tensor("xt", [P, 2 * NPC], f16, kind="ExternalInput")
    out = nc.dram_tensor("out", [2, P, NPC], f32, kind="ExternalOutput")

    with tile.TileContext(nc) as tc:
        with (
            tc.tile_pool(name="const", bufs=1) as cpool,
            tc.tile_pool(name="g", bufs=GBUFS) as gpool,
            tc.tile_pool(name="s", bufs=4) as spool,
            tc.tile_pool(name="pt", bufs=2, space="PSUM") as pt_pool,
            tc.tile_pool(name="po", bufs=2, space="PSUM") as po_pool,
        ):
            # critical loads first: gathers depend on idx table; split the
            # load so set-0 gathers start as soon as their slice lands
            NT0 = int(tile_ofs[8])
            idx_sb = cpool.tile([P, NT * 8], i16)
            nc.sync.dma_start(idx_sb[:, :NT0 * 8], idxsd.ap()[:, :NT0 * 8])
            nc.sync.dma_start(idx_sb[:, NT0 * 8:], idxsd.ap()[:, NT0 * 8:])
            slots_sb = cpool.tile([P, NT, 1], f16)
            nc.sync.dma_start(slots_sb[:], slotsd.ap())
            slotsb_sb = cpool.tile([P, NBT, 1], f16)
            nc.sync.dma_start(slotsb_sb[:], slotsbd.ap())
            iota_sb = cpool.tile([P, 8, P], f16)
            nc.sync.dma_start(iota_sb[:], iotad.ap())
            recip_sb = cpool.tile([P, SEGS], f16)
            nc.sync.dma_start(recip_sb[:], recipd.ap())
            w_sb = cpool.tile([P, R * 2 * 2 * P], f16)
            nc.sync.dma_start(w_sb[:], wt.ap())
            root_sb = cpool.tile([P, 2 * 2 * P], f16)
            nc.sync.dma_start(root_sb[:], roott.ap())
            bias_sb = cpool.tile([P, 2], f32)
            nc.sync.dma_start(bias_sb[:], biast.ap())
            xt_sb = cpool.tile([P, 2, NPC], f16)
            nc.sync.dma_start(xt_sb[:], xt.ap())

            mt = cpool.tile([P, 2, SEGS], f16)      # mean^T, all relations
            out_sb = cpool.tile([P, 2, NPC], f32)

            nc.gpsimd.load_library(library_config.mlp)

            g_tiles = {}
            s8_bufs = {}
            s8b_bufs = {}
            ndma = [0]

            def ensure_s8(tt):
                """Build the S one-hot batch containing global tile tt."""
                b = tt // 8
                if b in s8_bufs:
                    return
                n = min(8, NT - b * 8)
                s8 = spool.tile([P, n, P], f16, name=f"s8_{b}", tag="s8")
                nc.vector.tensor_tensor(
                    out=s8[:], in0=iota_sb[:, :n, :],
                    in1=slots_sb[:, b * 8:b * 8 + n, :]
                    .to_broadcast([P, n, P]),
                    op=mybir.AluOpType.is_equal)
                s8_bufs[b] = s8

            def ensure_s8b(bidx):
                b = bidx // 8
                if b in s8b_bufs:
                    return
                n = min(8, plan["nbt"] - b * 8)
                s8b = spool.tile([P, n, P], f16, name=f"s8b_{b}", tag="s8b")
                nc.vector.tensor_tensor(
                    out=s8b[:], in0=iota_sb[:, :n, :],
                    in1=slotsb_sb[:, b * 8:b * 8 + n, :]
                    .to_broadcast([P, n, P]),
                    op=mybir.AluOpType.is_equal)
                s8b_bufs[b] = s8b

            for p, gid in enumerate(PORDER):
                ntg = nt[p]
                g = gpool.tile([P, ntg, H], f16, name=f"g{p}", tag="g")
                # queue must follow the tile scheduler's global DMASW lane
                # rotation (lane = counter % 8, queue = lane % 4) so each
                # completion sem stays on one queue
                for t0 in range(0, ntg, GATHER_TILES):
                    n8 = min(GATHER_TILES, ntg - t0)
                    c0 = (int(tile_ofs[p]) + t0) * 8
                    nc.gpsimd.dma_gather(
                        g[:, t0:t0 + n8, :], xtab.ap(),
                        idx_sb[:, c0:c0 + n8 * 8],
                        num_idxs=n8 * P, num_idxs_reg=n8 * P, elem_size=H,
                        queue_num=ndma[0] % 4)
                    ndma[0] += 1
                g_tiles[p] = g

                psT = [pt_pool.tile([P, 512], f32, name=f"psT{p}_{kc}",
                                    tag=f"psT{kc}")
                       for kc in range(2)]
                for t in range(ntg):
                    tt = int(tile_ofs[p]) + t
                    ensure_s8(tt)
                    s8 = s8_bufs[tt // 8]
                    ti = tt % 8
                    bposA, stA, spA, bidx, bposB, stB, spB = tiles[p][t]
                    for kc in range(2):
                        nc.tensor.matmul(
                            out=psT[kc][:, bposA * P:(bposA + 1) * P],
                            lhsT=g[:, t, kc * P:(kc + 1) * P],
                            rhs=s8[:, ti, :],
                            start=stA, stop=spA)
                    if bidx >= 0:
                        ensure_s8b(bidx)
                        s8b = s8b_bufs[bidx // 8]
                        for kc in range(2):
                            nc.tensor.matmul(
                                out=psT[kc][:, bposB * P:(bposB + 1) * P],
                                lhsT=g[:, t, kc * P:(kc + 1) * P],
                                rhs=s8b[:, bidx % 8, :],
                                start=stB, stop=spB)
                # mean = sum * recip, fused into the PSUM->SBUF copy
                for kc in range(2):
                    nc.vector.tensor_tensor(
                        out=mt[:, kc, gid * 512:(gid + 1) * 512],
                        in0=psT[kc][:],
                        in1=recip_sb[:, gid * 512:(gid + 1) * 512],
                        op=mybir.AluOpType.mult)

                if p % 8 == 7:
                    # all groups of dst-chunk n4 are aggregated: dense GEMM
                    n4 = p // 8
                    for mc in range(2):
                        po = po_pool.tile([P, 512], f32, name=f"po{n4}_{mc}",
                                          tag="po")
                        for r in range(R):
                            for kc in range(2):
                                wofs = ((r * 2 + kc) * 2 + mc) * P
                                nc.tensor.matmul(
                                    out=po[:],
                                    lhsT=w_sb[:, wofs:wofs + P],
                                    rhs=mt[:, kc, r * NPC + n4 * 512:
                                           r * NPC + (n4 + 1) * 512],
                                    start=(r == 0 and kc == 0), stop=False)
                        for kc in range(2):
                            rofs = (kc * 2 + mc) * P
                            nc.tensor.matmul(
                                out=po[:],
                                lhsT=root_sb[:, rofs:rofs + P],
                                rhs=xt_sb[:, kc, n4 * 512:(n4 + 1) * 512],
                                start=False, stop=(kc == 1))
                        nc.scalar.add(
                            out=out_sb[:, mc, n4 * 512:(n4 + 1) * 512],
                            in_=po[:], add=bias_sb[:, mc:mc + 1])
                        nc.sync.dma_start(
                            out.ap()[mc][:, n4 * 512:(n4 + 1) * 512],
                            out_sb[:, mc, n4 * 512:(n4 + 1) * 512])

    nc.compile()
    return nc


def _prep_inputs(input_s, input_a, edge_index, edge_type, weight, root, bias):
    """Host-side sharding/layout prep. Returns (plan, in_maps)."""
    x = np.ascontiguousarray(
        np.concatenate([input_s, input_a], axis=1).reshape(N, H)
    ).astype(np.float32)
    xtab = x.astype(np.float16)

    src = np.asarray(edge_index[0]).astype(np.int64)
    dst = np.asarray(edge_index[1]).astype(np.int64)
    et = np.asarray(edge_type).astype(np.int64)

    cnt = np.bincount(dst * R + et, minlength=N * R).reshape(N, R)
    recip_full = (1.0 / np.maximum(cnt, 1)).astype(np.float32)  # [N, R]

    owner = dst // NPC
    ldst = dst - owner * NPC
    lseg = et * NPC + ldst                          # relation-major local seg
    block = lseg >> 7                               # 0..127
    gid = block >> 2
    gkey = np.empty(NGRP, np.int64)                 # gid -> processing pos
    for pos, g in enumerate(PORDER):
        gkey[g] = pos

    cnt_cb = np.bincount(owner * NBLK + block,
                         minlength=NCORES * NBLK).reshape(NCORES, NBLK)
    cnt_ub = cnt_cb.max(axis=0)                     # [128]
    plan = _plan(cnt_ub)

    # stream layout (uniform across cores): per processing position p,
    # blocks 4*gid..4*gid+3 at offsets block_ofs, padded to nt[p]*128
    nt = plan["nt"]
    tile_ofs = np.concatenate([[0], np.cumsum(nt)]).astype(int)
    NT = int(tile_ofs[-1])
    stream_len = NT * P
    # absolute stream offset of each block (per core-uniform)
    blk_abs = np.zeros(NBLK, np.int64)
    grp_pad_end = np.zeros(NGRP, np.int64)          # padded end per position
    for p, g in enumerate(PORDER):
        base = int(tile_ofs[p]) * P
        for i in range(4):
            blk_abs[4 * g + i] = base + plan["block_ofs"][p][i]
        grp_pad_end[p] = int(tile_ofs[p + 1]) * P

    # sort edges by (owner, block, src); relies on blocks being laid out in
    # processing order via blk_abs when scattering below
    order = np.lexsort((src, block, owner))
    s_src = src[order].astype(np.int16)
    s_slot = (lseg[order] & 127).astype(np.float16)
    s_owner = owner[order]
    s_block = block[order]
    # position within (owner, block) run
    key = s_owner * NBLK + s_block
    starts = np.concatenate([[0], np.cumsum(np.bincount(
        key, minlength=NCORES * NBLK))])
    pos_in_blk = np.arange(E) - starts[key]

    idxs = np.full((NCORES, stream_len), -1, np.int16)
    slots = np.full((NCORES, stream_len), -1.0, np.float16)
    dest = blk_abs[s_block] + pos_in_blk
    idxs[s_owner, dest] = s_src
    slots[s_owner, dest] = s_slot

    # boundary tiles: mask the second block's lanes out of the main slot
    # table (A) and move them into the compact B table
    nbt = plan["nbt"]
    slotsB = np.full((NCORES, max(nbt, 1), P), -1.0, np.float16)
    for p in range(NGRP):
        for t in range(nt[p]):
            bidx = plan["tiles"][p][t][3]
            if bidx < 0:
                continue
            bl = plan["bnd"][p][t]
            base = (int(tile_ofs[p]) + t) * P
            slotsB[:, bidx, bl:] = slots[:, base + bl:base + P]
            slots[:, base + bl:base + P] = -1.0

    # mid-stream pads must be valid HBM rows: forward-fill idx within each
    # core's stream, but keep trailing -1 (trimmed by the ucode) for group
    # tails of positions >= GBUFS (earlier positions first-touch the pool
    # buffers, so fill those too to avoid stale NaN garbage)
    # a group may use idx=-1 trailing pads (trimmed by the ucode: no
    # descriptors, no DMA) only if every lane it leaves unwritten was
    # already written by an earlier group in the same pool slot; otherwise
    # fill its pads so the whole extent is gathered (finite)
    # (pool slot assignment is scheduler-controlled, so the only safe
    # policy is to write every lane of every group)
    fill_full = np.ones(NGRP, bool)

    nvalid = np.zeros((NCORES, NGRP), np.int32)
    for c in range(NCORES):
        row = idxs[c].astype(np.int32)
        invalid = row < 0
        ff = np.where(invalid, 0, np.arange(stream_len))
        np.maximum.accumulate(ff, out=ff)
        filled = row[ff]
        keep_neg = np.zeros(stream_len, bool)
        for p in range(NGRP):
            end = int(grp_pad_end[p])
            beg = int(tile_ofs[p]) * P
            if fill_full[p]:
                nvalid[c, p] = end - beg
                continue
            # trailing invalid run of this group's stream segment
            seg = invalid[beg:end]
            nz = np.flatnonzero(~seg)
            tail_start = (beg + nz[-1] + 1) if len(nz) else beg
            keep_neg[tail_start:end] = True
            nvalid[c, p] = tail_start - beg
        row = np.where(invalid & ~keep_neg, filled, row)
        row[invalid & keep_neg] = -1
        idxs[c] = row.astype(np.int16)

    iota_host = np.tile(np.arange(P, dtype=np.float16), (P, 8, 1)
                        ).reshape(P, 8 * P)

    w_host = np.ascontiguousarray(
        np.asarray(weight, np.float32).reshape(R, 2, P, 2, P)
        .transpose(2, 0, 1, 3, 4).reshape(P, R * 2 * 2 * P)).astype(np.float16)
    root_host = np.ascontiguousarray(
        np.asarray(root, np.float32).reshape(2, P, 2, P)
        .transpose(1, 0, 2, 3).reshape(P, 2 * 2 * P)).astype(np.float16)
    bias_host = np.ascontiguousarray(
        np.asarray(bias, np.float32).reshape(2, P).T)

    in_maps = []
    for c in range(NCORES):
        xc = x[c * NPC:(c + 1) * NPC]              # [2048, 256]
        xt_host = np.ascontiguousarray(
            xc.T.reshape(2, P, NPC).transpose(1, 0, 2).reshape(P, 2 * NPC)
        ).astype(np.float16)
        idx_host = np.ascontiguousarray(
            np.tile(idxs[c].reshape(NT * 8, 16).T, (8, 1)))
        rc = recip_full[c * NPC:(c + 1) * NPC, :].T.reshape(SEGS)
        recip_host = np.ascontiguousarray(
            np.broadcast_to(rc.astype(np.float16), (P, SEGS)))
        in_maps.append({
            "xtab": xtab,
            "idxsd": idx_host,
            "slotsd": np.ascontiguousarray(slots[c].reshape(NT, P).T),
            "slotsbd": np.ascontiguousarray(slotsB[c].T),
            "cntd": np.ascontiguousarray(nvalid[c].reshape(1, NGRP)),
            "recipd": recip_host,
            "iotad": iota_host,
            "wt": w_host,
            "roott": root_host,
            "biast": bias_host,
            "xt": xt_host,
        })
    return plan, in_maps


def _run(in_maps, plan, trace=False, trace_cores=None):
    from concourse import bass_utils
    key = _meta_key(plan)
    if key not in _COMPILED:
        _COMPILED[key] = _build_program(plan)
    nc = _COMPILED[key]
    kwargs = {}
    if trace:
        _install_ntff_shim()
        bass_utils.upload_artifacts = lambda tmpdir: tmpdir
        kwargs = dict(trace=True,
                      trace_cores=trace_cores if trace_cores else [0])
    return bass_utils.run_bass_kernel_spmd(
        nc, in_maps, core_ids=list(range(NCORES)), **kwargs)


def _assemble(results):
    full = np.empty((N, H), np.float32)
    for c in range(NCORES):
        o = results[c]["out"]                      # [2, 128, 2048]
        full[c * NPC:(c + 1) * NPC, 0:P] = o[0].T
        full[c * NPC:(c + 1) * NPC, P:2 * P] = o[1].T
    dtrp = full.reshape(B, 2 * L, H)
    sent = np.ascontiguousarray(dtrp[:, :L, :])
    act = np.ascontiguousarray(dtrp[:, L:, :])
    return sent, act


def kernel(input_s, input_a, edge_index, edge_type, weight, root, bias,
           _trace=False, _trace_cores=None, _return_stats=False):
    plan, in_maps = _prep_inputs(input_s, input_a, edge_index, edge_type,
                                 weight, root, bias)
    res = _run(in_maps, plan, trace=_trace, trace_cores=_trace_cores)
    out = _assemble(res.results)
    if _return_stats:
        return out, res
    return out


def _install_ntff_shim():
    """Install antenv.axon_hooks NTFF profiling hook via ctypes (the agent
    image lacks the module; same mechanism trn_boot would use)."""
    import types, ctypes, contextlib
    if "antenv.axon_hooks" in sys.modules:
        return
    so_path = "/opt/axon/libaxon_pjrt.so"
    lib = ctypes.CDLL(so_path)
    if not hasattr(lib, "axon_start_nrt_profile"):
        return
    lib.axon_start_nrt_profile.argtypes = [ctypes.POINTER(ctypes.c_int64),
                                           ctypes.c_size_t]
    lib.axon_start_nrt_profile.restype = ctypes.c_int64
    lib.axon_stop_nrt_profile.argtypes = [ctypes.c_char_p]
    lib.axon_stop_nrt_profile.restype = ctypes.c_int64

    @contextlib.contextmanager
    def _hook(output_dir, device_ids):
        import jax
        jax.devices()
        if device_ids:
            ids = (ctypes.c_int64 * len(device_ids))(*device_ids)
            rc = lib.axon_start_nrt_profile(ids, len(device_ids))
        else:
            rc = lib.axon_start_nrt_profile(None, 0)
        if rc != 0:
            raise RuntimeError(f"axon_start_nrt_profile rc={rc}")
        try:
            yield
        finally:
            n = lib.axon_stop_nrt_profile(str(output_dir).encode())
            if n < 0:
                raise RuntimeError(f"axon_stop_nrt_profile rc={n}")

    import antenv
    mod = types.ModuleType("antenv.axon_hooks")
    mod.get_axon_ntff_profile_hook = lambda: _hook
    mod.set_axon_ntff_profile_hook = lambda h: None
    sys.modules["antenv.axon_hooks"] = mod
    antenv.axon_hooks = mod
